# revision 3
# baseline (speedup 1.0000x reference)
"""Contrastive soft-DTW loss kernel for Trainium2 (8 NeuronCores).

Fully on-device soft-DTW, 32 anchor/candidate pairs per core.

1) PE matmul builds the scaled cost matrix D~ = D/gamma per pair via one
   augmented matmul (K=66), evicted raw to an internal DRAM scratch in a
   row-gatherable layout.
2) The 399-row DP runs in a numerically safe exp-domain form: per row, a
   min-plus (tropical) scan computes the exact hard-DTW row M[j]
   (tensor_tensor_scan op0=min/op1=add); softmin exponents are taken
   relative to M, which provably bounds the exp-scan state
   s = e^{-(R~-M)} in [1, ~3T] so fp32 never under/overflows:
     s[j] = eP[j]*s[j-1] + w[j]     (tensor_tensor_scan mult/add)
     R~[i,j] = M[j] - ln s[j]
3) Chunked wavefront: the 400 columns are split into 4 chunks of 100 laid
   out on 128 partitions (32 pairs x 4 chunks), chunk c lagging chunk c-1
   by one row. Every step advances one row per chunk at [128,100] op
   granularity. Chunk boundaries (tropical M and soft R~) travel to the
   next partition group through tiny PE permutation matmuls; the
   horizontal softmin entry of a chunk folds into an extra w-term built
   from the same-row R~ boundary, so no s-boundary is needed.
4) Per step, R~[row, lb_p] is extracted with a one-hot masked reduction
   (scalar_tensor_tensor accum_out); the host picks the (chunk, step) of
   (la_p, lb_p) per pair and finishes the tiny contrastive reduction.

Host fallback (pure numpy, same algorithm) guards against device failure.
"""

import os

import numpy as np

LAST_RESULTS = None  # BassKernelResults of the last device run (for test.py)

NW, NG, NF = 16, 5, 10
STEP = 1 + NG + NF          # 16
T, DIM = 400, 64
GAMMA = 5.0
BIG = 1e10
BIGS = float(np.float32(BIG / GAMMA))
NCORES = 8
PPC = (NW * STEP) // NCORES  # 32 pairs per core
WPC = PPC // STEP            # 2 writers per core
KAUG = DIM + 2               # 66
CH = 4                       # column chunks
CW = T // CH                 # 100 columns per chunk
NROW = T - 1                 # DP rows 2..400
NST = NROW + CH - 1          # wavefront steps

_BLOCKS = []
_c = 1
while _c <= T - 1:
    _nb = min(128, T - _c)
    _BLOCKS.append((_c, _nb))
    _c += _nb


def _patch_drain():
    """Split the tile-context teardown Drain's semaphore waits across
    separate sync-engine nops (this walrus rejects multi-wait Drains)."""
    import concourse.tile as tile
    from concourse import mybir
    from concourse.vector_clock import ScopedClock

    if getattr(tile.TileContext, "_drain_patched", False):
        return
    MAXW = 1

    def _drain_and_barrier(self, tick_clock, wait_clock):
        nc = self.nc
        probe = nc.sync.nop(nofuse=True)
        wait_clock.add_sem_waits(
            probe.ins, ScopedClock({None: tick_clock.global_clock})
        )
        si = probe.ins.sync_info
        waits = list(si.on_wait) if si is not None else []
        ups = list(si.on_update) if si is not None else []
        if len(waits) > MAXW:
            probe.ins.sync_info = mybir.SyncInfo(on_wait=waits[:MAXW], on_update=ups)
            rest = waits[MAXW:]
            for k in range(0, len(rest), MAXW):
                n = nc.sync.nop(nofuse=True)
                n.ins.sync_info = mybir.SyncInfo(
                    on_wait=rest[k:k + MAXW], on_update=[]
                )
        nc.sync.drain()
        nc.all_engine_barrier()
        assert self.sems is not None
        popped = nc._tile_sem_poison_stack.pop()
        assert popped is self._sem_poison
        nc.clear_and_free_semaphores(list(self.sems.allocated().values()))
        nc.all_engine_barrier()

    tile.TileContext._drain_and_barrier = _drain_and_barrier
    tile.TileContext._drain_patched = True


def _split_bir_waits(bir_bytes):
    """This walrus rejects engine instructions carrying more than one
    embedded sync-wait. Hoist all but one wait of every instruction onto
    injected same-engine NoOps placed just before it."""
    import json

    bir = json.loads(bir_bytes)
    ctr = [0]

    def fix_block(insts):
        out = []
        for ins in insts:
            si = ins.get("sync_info")
            waits = (si or {}).get("on_wait") or []
            if len(waits) > 1:
                for wv in waits[:-1]:
                    ctr[0] += 1
                    out.append({
                        "debug": ins.get("debug", 0),
                        "engine": ins["engine"],
                        "ins": [], "outs": [],
                        "name": f"I-SW{ctr[0]}",
                        "opcode": "NoOp",
                        "sync_info": {"on_update": [], "on_wait": [wv]},
                    })
                si["on_wait"] = [waits[-1]]
            out.append(ins)
        return out

    def walk(o):
        if isinstance(o, dict):
            if isinstance(o.get("instructions"), list):
                o["instructions"] = fix_block(o["instructions"])
            for v in o.values():
                walk(v)
        elif isinstance(o, list):
            for v in o:
                walk(v)

    walk(bir)
    return json.dumps(bir).encode()


def _patch_compile():
    from concourse import bass2jax

    if getattr(bass2jax, "_split_waits_patched", False):
        return
    orig = bass2jax.compile_bir_kernel

    def wrapped(bir, *a, **k):
        return orig(_split_bir_waits(bir), *a, **k)

    bass2jax.compile_bir_kernel = wrapped
    bass2jax._split_waits_patched = True


def _active(u):
    """Active chunk range [cmin, cmax] at wavefront step u."""
    return max(0, u - (NROW - 1)), min(CH - 1, u)


def _build_bass():
    import concourse.bass as bass
    import concourse.tile as tile
    from concourse import mybir

    _patch_drain()
    _patch_compile()
    f32 = mybir.dt.float32
    op = mybir.AluOpType
    act = mybir.ActivationFunctionType

    nc = bass.Bass()
    lhsT = nc.dram_tensor("lhsT", [WPC, KAUG, T], f32, kind="ExternalInput")
    rhs = nc.dram_tensor("rhs", [PPC, KAUG, T], f32, kind="ExternalInput")
    pre0 = nc.dram_tensor("pre0", [128, CW + 1], f32, kind="ExternalInput")
    pre1 = nc.dram_tensor("pre1", [128, CW + 1], f32, kind="ExternalInput")
    hselc = nc.dram_tensor("hselc", [128, CW], f32, kind="ExternalInput")
    pshift = nc.dram_tensor("pshift", [128, 128], f32, kind="ExternalInput")
    rsel2 = nc.dram_tensor("rsel2", [128, NST], f32, kind="ExternalOutput")
    adram = nc.dram_tensor("adram", [NROW, PPC, T], f32, kind="Internal")

    with tile.TileContext(nc) as tc:
        with tc.tile_pool(name="wp", bufs=1) as wp, \
             tc.tile_pool(name="rh", bufs=3) as rh, \
             tc.tile_pool(name="mmp", bufs=3, space="PSUM") as mmp, \
             tc.tile_pool(name="ev", bufs=4) as evp, \
             tc.tile_pool(name="bp", bufs=2, space="PSUM") as bpp, \
             tc.tile_pool(name="row", bufs=8) as rowp, \
             tc.tile_pool(name="st", bufs=3) as stp, \
             tc.tile_pool(name="pp", bufs=1) as pp:

            # --- persistent state ---
            lt = []
            for w in range(WPC):
                t_ = wp.tile([KAUG, T], f32, tag=f"lt{w}")
                nc.sync.dma_start(out=t_, in_=lhsT[w])
                lt.append(t_)
            B = [pp.tile([128, CW + 1], f32, tag="B0", name="B0"),
                 pp.tile([128, CW + 1], f32, tag="B1", name="B1")]
            nc.sync.dma_start(out=B[0], in_=pre0[:, :])
            nc.sync.dma_start(out=B[1], in_=pre1[:, :])
            hs = pp.tile([128, CW], f32, tag="hs")
            nc.sync.dma_start(out=hs, in_=hselc[:, :])
            psh = pp.tile([128, 128], f32, tag="psh")
            nc.sync.dma_start(out=psh, in_=pshift[:, :])
            M = pp.tile([128, CW + 1], f32, tag="M")
            nc.vector.memset(M[0:32, 0:1], BIGS)
            rstore = pp.tile([128, NST], f32, tag="rstore")
            # border-constant injector for the boundary shuttles: a second
            # accumulating matmul adds BIGS into rows 0..31 (chunk 0's
            # tropical/soft left border), so landings can copy all 128
            # partitions (walrus rejects offset PSUM reads > 32 partitions).
            biasrow = pp.tile([1, 128], f32, tag="biasrow")
            nc.vector.memset(biasrow[0:1, 0:32], BIGS)
            nc.vector.memset(biasrow[0:1, 32:128], 0.0)
            ones1 = pp.tile([1, 1], f32, tag="ones1")
            nc.vector.memset(ones1, 1.0)

            # --- cost-matrix phase: D~ rows 1..399 to adram ---
            for p in range(PPC):
                rt = rh.tile([KAUG, T], f32, tag="rt")
                nc.sync.dma_start(out=rt, in_=rhs[p])
                w = p // STEP
                for (c0, nb) in _BLOCKS:
                    ps = mmp.tile([128, T], f32, tag="ps")
                    nc.tensor.matmul(ps[:nb], lt[w][:, c0:c0 + nb], rt[:, :],
                                     start=True, stop=True)
                    ev = evp.tile([128, T], f32, tag="ev")
                    nc.scalar.copy(out=ev[:nb], in_=ps[:nb])
                    nc.sync.dma_start(out=adram[c0 - 1:c0 - 1 + nb, p],
                                      in_=ev[:nb])

            # --- chunked wavefront row DP ---
            psM_prev = psR_prev = None
            for u in range(NST):
                cmin, cmax = _active(u)
                # all compute spans [0:phi]: partition offsets != 0 are
                # limited to 32 partitions by this walrus, and garbage in
                # warm-up/drained chunks is provably inert (BIGS borders /
                # retired before any consumer).
                plo, phi = 0, 32 * cmax + 32
                cur, nxt = B[u % 2], B[(u + 1) % 2]

                drow = rowp.tile([128, CW], f32, tag="drow")
                for c in range(cmin, cmax + 1):
                    nc.sync.dma_start(
                        out=drow[32 * c:32 * c + 32, :],
                        in_=adram[u - c, :, c * CW:(c + 1) * CW])

                # land previous step's boundary shuttles (full range; rows
                # 0..31 receive the matmul-injected BIGS border)
                if psM_prev is not None:
                    nc.scalar.copy(out=M[:, 0:1], in_=psM_prev[:, 0:1])
                    nc.scalar.copy(out=nxt[:, 0:1], in_=psR_prev[:, 0:1])

                mn = stp.tile([128, CW], f32, tag="mn")
                nc.vector.tensor_tensor(
                    out=mn[plo:phi], in0=cur[plo:phi, 0:CW],
                    in1=cur[plo:phi, 1:CW + 1], op=op.min)
                # M[k] = min(mn[k], M[k-1]) + drow[k]
                nc.vector.tensor_tensor_scan(
                    out=M[plo:phi, 1:CW + 1], data0=mn[plo:phi],
                    data1=drow[plo:phi], initial=M[plo:phi, 0:1],
                    op0=op.min, op1=op.add)
                psM = bpp.tile([128, 1], f32, tag="psM")
                nc.tensor.matmul(psM, psh[:, :], M[:, CW:CW + 1],
                                 start=True, stop=False)
                nc.tensor.matmul(psM, biasrow[:, :], ones1[:, :],
                                 start=False, stop=True)

                dpm = stp.tile([128, CW], f32, tag="dpm")
                nc.vector.tensor_tensor(out=dpm[plo:phi], in0=drow[plo:phi],
                                        in1=M[plo:phi, 1:CW + 1],
                                        op=op.subtract)
                tst = stp.tile([128, 3 * CW + 1], f32, tag="tst")
                nc.vector.tensor_tensor(
                    out=tst[plo:phi, 2 * CW:3 * CW], in0=M[plo:phi, 0:CW],
                    in1=dpm[plo:phi], op=op.add)
                # horizontal entry: t_extra = R~bnd + drow[0] - M[1]
                # (rows 0..31: R~bnd is the BIGS border => exp underflows
                # to exactly 0, the correct no-horizontal-entry semantics)
                tx = stp.tile([128, 1], f32, tag="tx")
                nc.vector.tensor_tensor(
                    out=tx[plo:phi], in0=nxt[plo:phi, 0:1],
                    in1=M[plo:phi, 1:2], op=op.subtract)
                nc.vector.tensor_tensor(
                    out=tst[plo:phi, 3 * CW:3 * CW + 1], in0=tx[plo:phi],
                    in1=drow[plo:phi, 0:1], op=op.add)
                est = stp.tile([128, 3 * CW + 1], f32, tag="est")
                nc.scalar.activation(
                    out=est[plo:phi, 2 * CW:3 * CW + 1],
                    in_=tst[plo:phi, 2 * CW:3 * CW + 1],
                    func=act.Exp, scale=-1.0)
                nc.vector.tensor_tensor(out=tst[plo:phi, 0:CW],
                                        in0=cur[plo:phi, 0:CW],
                                        in1=dpm[plo:phi], op=op.add)
                nc.vector.tensor_tensor(out=tst[plo:phi, CW:2 * CW],
                                        in0=cur[plo:phi, 1:CW + 1],
                                        in1=dpm[plo:phi], op=op.add)
                nc.scalar.activation(
                    out=est[plo:phi, 0:2 * CW], in_=tst[plo:phi, 0:2 * CW],
                    func=act.Exp, scale=-1.0)
                wt = stp.tile([128, CW], f32, tag="wt")
                nc.vector.tensor_tensor(out=wt[plo:phi],
                                        in0=est[plo:phi, 0:CW],
                                        in1=est[plo:phi, CW:2 * CW],
                                        op=op.add)
                nc.vector.tensor_tensor(
                    out=wt[plo:phi, 0:1], in0=wt[plo:phi, 0:1],
                    in1=est[plo:phi, 3 * CW:3 * CW + 1], op=op.add)
                st = stp.tile([128, CW], f32, tag="st")
                nc.vector.tensor_tensor_scan(
                    out=st[plo:phi], data0=est[plo:phi, 2 * CW:3 * CW],
                    data1=wt[plo:phi], initial=0.0, op0=op.mult, op1=op.add)
                ln = stp.tile([128, CW], f32, tag="ln")
                nc.scalar.activation(out=ln[plo:phi], in_=st[plo:phi],
                                     func=act.Ln, scale=1.0)
                nc.vector.tensor_tensor(
                    out=nxt[plo:phi, 1:CW + 1], in0=M[plo:phi, 1:CW + 1],
                    in1=ln[plo:phi], op=op.subtract)
                psR = bpp.tile([128, 1], f32, tag="psR")
                nc.tensor.matmul(psR, psh[:, :], nxt[:, CW:CW + 1],
                                 start=True, stop=False)
                nc.tensor.matmul(psR, biasrow[:, :], ones1[:, :],
                                 start=False, stop=True)
                sc = stp.tile([128, CW], f32, tag="sc")
                nc.vector.scalar_tensor_tensor(
                    out=sc[plo:phi], in0=hs[plo:phi], scalar=1.0,
                    in1=nxt[plo:phi, 1:CW + 1], op0=op.mult, op1=op.mult,
                    accum_out=rstore[plo:phi, u:u + 1])
                psM_prev, psR_prev = psM, psR

            nc.sync.dma_start(out=rsel2[:, :], in_=rstore)
    return nc


def _prep_inputs(A, B, la, lb):
    """Build per-core input maps. A/B: [256, T, DIM] fp32."""
    P = A.shape[0]
    asq = np.sum(A * A, axis=-1)
    bsq = np.sum(B * B, axis=-1)

    anchors = A[::STEP]
    asq_w = asq[::STEP]
    lhsT = np.empty((NW, KAUG, T), np.float32)
    lhsT[:, :DIM] = np.transpose(anchors * np.float32(-2.0 / GAMMA), (0, 2, 1))
    lhsT[:, DIM] = asq_w / np.float32(GAMMA)
    lhsT[:, DIM + 1] = 1.0

    rhs = np.empty((P, KAUG, T), np.float32)
    rhs[:, :DIM] = np.transpose(B, (0, 2, 1))
    rhs[:, DIM] = 1.0
    rhs[:, DIM + 1] = bsq / np.float32(GAMMA)

    # row 1 of the DP: R~[1, j] = cumsum_j D~[0, j-1]
    d0 = (asq[:, 0:1] + bsq - 2.0 * np.einsum("pd,ptd->pt", A[:, 0], B)) \
        / np.float32(GAMMA)
    r1 = np.empty((P, T + 1), np.float32)
    r1[:, 0] = BIGS
    r1[:, 1:] = np.cumsum(d0.astype(np.float32), axis=1, dtype=np.float32)

    # chunked row-1 preloads (parity c%2) and one-hot lb masks
    pshift = np.zeros((128, 128), np.float32)
    for k in range(96):
        pshift[k, k + 32] = 1.0

    in_maps = []
    for core in range(NCORES):
        sl = slice(core * PPC, (core + 1) * PPC)
        wsl = slice(core * WPC, (core + 1) * WPC)
        r1c = r1[sl]
        pre = [np.full((128, CW + 1), BIGS, np.float32) for _ in range(2)]
        for c in range(CH):
            dst = pre[c % 2]
            dst[32 * c:32 * c + 32, 0] = r1c[:, c * CW]
            dst[32 * c:32 * c + 32, 1:] = r1c[:, c * CW + 1:(c + 1) * CW + 1]
        hselc = np.zeros((128, CW), np.float32)
        lbc = lb[sl]
        cstar = (lbc - 1) // CW
        kstar = lbc - cstar * CW            # 1..CW
        hselc[32 * cstar + np.arange(PPC), kstar - 1] = 1.0
        in_maps.append({
            "lhsT": np.ascontiguousarray(lhsT[wsl]),
            "rhs": np.ascontiguousarray(rhs[sl]),
            "pre0": pre[0], "pre1": pre[1],
            "hselc": hselc,
            "pshift": pshift,
        })
    return in_maps


def _device_r(A, B, la, lb):
    from concourse.bass_utils import run_bass_kernel_spmd

    in_maps = _prep_inputs(A, B, la, lb)
    nc = _build_bass()
    kw = {}
    if os.environ.get("KERNEL_TRACE", "") == "1":
        kw = dict(trace=True, tmpdir=os.environ.get("KERNEL_TRACE_DIR") or None)
    res = run_bass_kernel_spmd(nc, in_maps, core_ids=list(range(NCORES)), **kw)
    global LAST_RESULTS
    LAST_RESULTS = res
    r = np.empty(A.shape[0], np.float32)
    for core in range(NCORES):
        out = res.results[core]["rsel2"]          # [128, NST]
        sl = slice(core * PPC, (core + 1) * PPC)
        lbc, lac = lb[sl], la[sl]
        cstar = (lbc - 1) // CW
        part = 32 * cstar + np.arange(PPC)
        step = lac + cstar - 2
        r[sl] = np.float32(GAMMA) * out[part, step]
    return r


# ---------------- host fallback (same algorithm, numpy) ----------------

def _host_r(A, B, la, lb):
    P = A.shape[0]
    asq = np.sum(A * A, axis=-1)
    bsq = np.sum(B * B, axis=-1)
    cross = np.einsum("ptd,psd->pts", A, B, optimize=True)
    Dt = ((asq[:, :, None] + bsq[:, None, :] - 2.0 * cross)
          / np.float32(GAMMA)).astype(np.float32)
    Rp = np.empty((P, T + 1), np.float32)
    Rp[:, 0] = BIGS
    Rp[:, 1:] = np.cumsum(Dt[:, 0, :], axis=1, dtype=np.float32)
    rsel = np.zeros((P, T + 1), np.float32)
    rsel[:, 1] = Rp[np.arange(P), lb]
    M = np.empty((P, T + 1), np.float32)
    M[:, 0] = BIGS
    Rn = np.empty_like(Rp)
    Rn[:, 0] = BIGS
    for i in range(2, T + 1):
        Drow = Dt[:, i - 1, :]
        vh = np.minimum(Rp[:, :-1], Rp[:, 1:]) + Drow
        c = np.full(P, BIGS, np.float32)
        for j in range(T):
            c = np.minimum((Drow[:, j] + c).astype(np.float32), vh[:, j])
            M[:, j + 1] = c
        dpm = Drow - M[:, 1:]
        e1 = np.exp(-(Rp[:, :-1] + dpm))
        e2 = np.exp(-(Rp[:, 1:] + dpm))
        eP = np.exp(-(M[:, :-1] + dpm))
        w = (e1 + e2).astype(np.float32)
        c = np.zeros(P, np.float32)
        s = np.empty((P, T), np.float32)
        for j in range(T):
            c = (eP[:, j] * c + w[:, j]).astype(np.float32)
            s[:, j] = c
        Rn[:, 1:] = M[:, 1:] - np.log(s)
        rsel[:, i] = Rn[np.arange(P), lb]
        Rp, Rn = Rn, Rp
    return np.float32(GAMMA) * rsel[np.arange(P), la]


def kernel(data, margin, lens):
    data = np.asarray(data, dtype=np.float32)
    margin = np.asarray(margin, dtype=np.float32)
    lens = np.asarray(lens)

    batch = data.reshape(NW, STEP, T, DIM)
    blens = lens.reshape(NW, STEP)
    A = np.ascontiguousarray(
        np.broadcast_to(batch[:, :1], batch.shape).reshape(NW * STEP, T, DIM),
        dtype=np.float32)
    B = np.ascontiguousarray(batch.reshape(NW * STEP, T, DIM),
                             dtype=np.float32)
    la = np.broadcast_to(blens[:, :1], blens.shape).reshape(-1).astype(np.int64)
    lb = blens.reshape(-1).astype(np.int64)

    try:
        r = _device_r(A, B, la, lb)
    except Exception as e:  # pragma: no cover - device fallback
        import traceback
        traceback.print_exc()
        print(f"[kernel] device path failed ({e!r}); falling back to host")
        r = _host_r(A, B, la, lb)

    dists = (r / (la + lb).astype(np.float32)).reshape(NW, STEP)
    dist_aa = dists[:, 0:1]
    lk1 = dists[:, 1:1 + NG] - dist_aa
    lk2 = np.maximum(margin[0] - (dists[:, 1 + NG:1 + NG + NF] - dist_aa), 0.0)
    nz = (lk1 != 0).sum(axis=1) + (lk2 != 0).sum(axis=1) + 1
    lv = (lk1.sum(axis=1) + lk2.sum(axis=1)) / nz
    return np.float32(lv.mean())

